# revision 28
# baseline (speedup 1.0000x reference)
# Multi-head attention (B=4, S=2048, D=1024, H=16) on 8 TRN2 NeuronCores.
#
# Sharding: core c handles batch b = c//2 and head-group g = c%2 (8 heads).
# Each core computes Q/K/V projections for its 8 heads, full attention over
# S=2048, and a partial output projection over its 512 value-features.
# Host sums the two partial outputs per batch and adds b_out.
#
# Device math (per core, all matmuls bf16 with fp32 PSUM accumulation):
#   Q^T, K^T  [512, S]   = W_slice @ x^T (+ per-partition bias)
#   V         [S, 512]   = x @ Wv^T  (bias folded into values, see below)
#   S^T tiles [128k, q]  = K_h Q_h^T, two heads row-tiled on the PE array
#   P^T       = exp(S^T * 0.125)                 (ACT engine, bf16 out)
#   [PV^T; l] = [V | 1]^T @ P^T                  (l = softmax denominator)
#   values^T  = PV^T * (1/l) + b_v               (exact: (P(V+b))/l = PV/l + b)
#   out^T     = W_out_slice^T-contraction over 512 features (partial)
import sys

for _p in ("/opt/trn_rl_repo",):
    if _p not in sys.path:
        sys.path.insert(0, _p)

import numpy as np
import ml_dtypes

BF16 = ml_dtypes.bfloat16

B, S, D = 4, 2048, 1024
H, HD = 16, 64
P = 128
HC = 8            # heads per core
DC = HC * HD      # 512 value-features per core
KS = D // P       # 8 contraction subtiles for projections
MT = DC // P      # 4 M-tiles == head pairs
ST = S // P       # 16 seq tiles
NQ = 512          # q-chunk (matmul free dim)
QC = S // NQ      # 4
OT = D // P       # 8 output e-tiles
OKS = DC // P     # 4 contraction subtiles for out-proj


def build_nc(s=S):
    """Build the single-core Bass/Tile program (SPMD across 8 cores)."""
    import concourse.tile as tile
    import concourse.mybir as mybir
    from concourse import bacc
    from contextlib import ExitStack

    dt = mybir.dt
    st_n = s // P
    qc_n = s // NQ

    nc = bacc.Bacc("TRN2", target_bir_lowering=False, debug=False)

    xT = nc.dram_tensor("xT", [P, KS, s], dt.bfloat16, kind="ExternalInput").ap()
    wq = nc.dram_tensor("wq", [P, KS, DC], dt.bfloat16, kind="ExternalInput").ap()
    wk = nc.dram_tensor("wk", [P, KS, DC], dt.bfloat16, kind="ExternalInput").ap()
    wv = nc.dram_tensor("wv", [P, KS, DC], dt.bfloat16, kind="ExternalInput").ap()
    wo = nc.dram_tensor("wo", [P, OKS, D], dt.bfloat16, kind="ExternalInput").ap()
    bq = nc.dram_tensor("bq", [P, MT], dt.float32, kind="ExternalInput").ap()
    bk = nc.dram_tensor("bk", [P, MT], dt.float32, kind="ExternalInput").ap()
    bv = nc.dram_tensor("bv", [P, MT], dt.float32, kind="ExternalInput").ap()
    out = nc.dram_tensor("out", [P, OT, s], dt.float32, kind="ExternalOutput").ap()

    with tile.TileContext(nc) as tc, ExitStack() as ctx:
        persist = ctx.enter_context(tc.tile_pool(name="persist", bufs=1))
        # PSUM budget is 8 banks: scores 2x[128,1024] (4) + one shared 4-slot
        # pool of 1-bank tiles for proj/pv/outproj accumulators (4).
        ps_s = ctx.enter_context(tc.tile_pool(name="ps_s", bufs=2, space="PSUM"))
        ps_pv = ctx.enter_context(tc.tile_pool(name="ps_pv", bufs=4, space="PSUM"))
        ps_io = ps_pv
        ptp = ctx.enter_context(tc.tile_pool(name="ptp", bufs=4))
        small = ctx.enter_context(tc.tile_pool(name="small", bufs=4))
        dramp = ctx.enter_context(tc.tile_pool(name="dramp", bufs=4, space="DRAM"))

        xT_sb = persist.tile([P, KS, s], dt.bfloat16, name="xT_sb")
        wq_sb = persist.tile([P, KS, DC], dt.bfloat16, name="wq_sb")
        wk_sb = persist.tile([P, KS, DC], dt.bfloat16, name="wk_sb")
        wv_sb = persist.tile([P, KS, DC], dt.bfloat16, name="wv_sb")
        wo_sb = persist.tile([P, OKS, D], dt.bfloat16, name="wo_sb")
        bq_sb = persist.tile([P, MT], dt.float32, name="bq_sb")
        bk_sb = persist.tile([P, MT], dt.float32, name="bk_sb")
        bv_sb = persist.tile([P, MT], dt.float32, name="bv_sb")
        QT_sb = persist.tile([P, MT, s], dt.bfloat16, name="QT_sb")
        KT_sb = persist.tile([P, MT, s], dt.bfloat16, name="KT_sb")
        V_sb = persist.tile([P, st_n, HC, HD + 1], dt.bfloat16, name="V_sb")
        VL_sb = persist.tile([P, OKS, s], dt.bfloat16, name="VL_sb")

        nc.sync.dma_start(out=wk_sb, in_=wk)
        for ks in range(KS):
            nc.sync.dma_start(out=xT_sb[:, ks], in_=xT[:, ks])
        nc.sync.dma_start(out=wq_sb, in_=wq)
        nc.sync.dma_start(out=bq_sb, in_=bq)
        nc.sync.dma_start(out=bk_sb, in_=bk)
        nc.sync.dma_start(out=wv_sb, in_=wv)
        nc.sync.dma_start(out=bv_sb, in_=bv)
        nc.sync.dma_start(out=wo_sb, in_=wo)
        # ones column for the fused softmax-denominator trick
        nc.vector.memset(V_sb[:, :, :, HD:HD + 1], 1.0)

        def proj_qk_chunk(w_sb, b_sb, dst, mt, c, wch):
            # dst[:, mt, c-chunk] = W_slice @ x^T + bias (128 rows = 2 heads)
            ps = ps_io.tile([P, NQ], dt.float32, name=f"ps_{wch}_{mt}_{c}", tag="pv")
            for ks in range(KS):
                nc.tensor.matmul(
                    ps,
                    lhsT=w_sb[:, ks, mt * P:(mt + 1) * P],
                    rhs=xT_sb[:, ks, c * NQ:(c + 1) * NQ],
                    start=(ks == 0),
                    stop=(ks == KS - 1),
                )
            nc.vector.tensor_add(
                dst[:, mt, c * NQ:(c + 1) * NQ],
                ps,
                b_sb[:, mt:mt + 1].to_broadcast((P, NQ)),
            )

        def proj_qk(w_sb, b_sb, dst, mt, wch):
            for c in range(qc_n):
                proj_qk_chunk(w_sb, b_sb, dst, mt, c, wch)

        def proj_v(st):
            # V[s-tile, :] = x @ Wv^T (no bias here; folded into values)
            ps = ps_io.tile([P, DC], dt.float32, name=f"ps_v_{st}", tag="pv")
            for ks in range(KS):
                nc.tensor.matmul(
                    ps,
                    lhsT=xT_sb[:, ks, st * P:(st + 1) * P],
                    rhs=wv_sb[:, ks, :],
                    start=(ks == 0),
                    stop=(ks == KS - 1),
                )
            nc.vector.tensor_copy(
                out=V_sb[:, st, :, 0:HD],
                in_=ps.rearrange("p (h d) -> p h d", h=HC),
            )

        def attention_pair(pr, filler=None):
            # filler(c, kt) emits interleaved PE work (projections for the
            # next pair, V tiles, out-proj chunks) so the ACT-bound kt loop
            # keeps the PE's slack cycles productive.
            for c in range(qc_n):
                cs = slice(c * NQ, (c + 1) * NQ)
                pvA = ps_pv.tile([HD + 1, NQ], dt.float32, name=f"pvA_{pr}_{c}", tag="pv")
                pvB = ps_pv.tile([HD + 1, NQ], dt.float32, name=f"pvB_{pr}_{c}", tag="pv")
                for kt in range(st_n):
                    if filler is not None:
                        filler(c, kt)
                    ks_sl = slice(kt * P, (kt + 1) * P)
                    ss = ps_s.tile([P, 2 * NQ], dt.float32, name=f"ss_{pr}_{c}_{kt}", tag="ss")
                    # S^T = K_h Q_h^T for the two heads, row-tiled (K=64 each)
                    nc.tensor.matmul(
                        ss[:, 0:NQ],
                        lhsT=KT_sb[0:HD, pr, ks_sl],
                        rhs=QT_sb[0:HD, pr, cs],
                    )
                    nc.tensor.matmul(
                        ss[:, NQ:2 * NQ],
                        lhsT=KT_sb[HD:P, pr, ks_sl],
                        rhs=QT_sb[HD:P, pr, cs],
                    )
                    pt = ptp.tile([P, 2 * NQ], dt.bfloat16, name=f"pt_{pr}_{c}_{kt}", tag="pt")
                    nc.scalar.activation(
                        pt, ss, mybir.ActivationFunctionType.Exp, scale=0.125
                    )
                    nc.tensor.matmul(
                        pvA,
                        lhsT=V_sb[:, kt, 2 * pr, :],
                        rhs=pt[:, 0:NQ],
                        start=(kt == 0),
                        stop=(kt == st_n - 1),
                    )
                    nc.tensor.matmul(
                        pvB,
                        lhsT=V_sb[:, kt, 2 * pr + 1, :],
                        rhs=pt[:, NQ:2 * NQ],
                        start=(kt == 0),
                        stop=(kt == st_n - 1),
                    )
                # Normalize: values^T = PV^T * (1/l) + b_v; head B is DMA-shifted
                # to partitions 64..127 so out-proj sees [128, s] rhs tiles.
                for half, pv in ((0, pvA), (1, pvB)):
                    # reciprocal of the l row (psum partition 64 -> sbuf p64),
                    # then DMA-broadcast it down to partitions 0..63
                    rec = small.tile([HD + 1, NQ], dt.float32, name=f"r_{pr}_{c}_{half}", tag="rec")
                    nc.vector.reciprocal(rec[HD:HD + 1, :], pv[HD:HD + 1, :])
                    # SBUF APs can't have stride-0 partition dims, DRAM APs can:
                    # bounce the reciprocal row through DRAM to broadcast it.
                    ld = dramp.tile([1, NQ], dt.float32, name=f"ld_{pr}_{c}_{half}", tag="ld")
                    nc.sync.dma_start(out=ld, in_=rec[HD:HD + 1, :])
                    bc = small.tile([HD, NQ], dt.float32, name=f"bc_{pr}_{c}_{half}", tag="bc")
                    nc.sync.dma_start(out=bc, in_=ld.to_broadcast((HD, NQ)))
                    if half == 0:
                        nc.vector.tensor_mul(VL_sb[0:HD, pr, cs], pv[0:HD, :], bc)
                    else:
                        stg = small.tile([HD, NQ], dt.bfloat16, name=f"st_{pr}_{c}", tag="stg")
                        nc.vector.tensor_mul(stg, pv[0:HD, :], bc)
                        nc.sync.dma_start(out=VL_sb[HD:P, pr, cs], in_=stg)
                nc.vector.tensor_add(
                    VL_sb[:, pr, cs],
                    VL_sb[:, pr, cs],
                    bv_sb[:, pr:pr + 1].to_broadcast((P, NQ)),
                )

        def outproj_chunk(c):
            # partial out-projection over this core's 512 value-features
            for et in range(OT):
                po = ps_io.tile([P, NQ], dt.float32, name=f"po_{et}_{c}", tag="pv")
                for ks in range(OKS):
                    nc.tensor.matmul(
                        po,
                        lhsT=wo_sb[:, ks, et * P:(et + 1) * P],
                        rhs=VL_sb[:, ks, c * NQ:(c + 1) * NQ],
                        start=(ks == 0),
                        stop=(ks == OKS - 1),
                    )
                ot_sb = small.tile([P, NQ], dt.float32, name=f"ot_{et}_{c}", tag="ot")
                nc.vector.tensor_copy(out=ot_sb, in_=po)
                nc.sync.dma_start(out=out[:, et, c * NQ:(c + 1) * NQ], in_=ot_sb)

        # Interleaved emission. Pair 0: K0 fully + Q0 chunk 0 up front; V
        # tiles stream inside chunk 0's kt loop; later Q0 chunks mid-chunk.
        proj_qk(wk_sb, bk_sb, KT_sb, 0, "k")
        proj_qk_chunk(wq_sb, bq_sb, QT_sb, 0, 0, "q")

        f1, f2, fmid = st_n // 4, (3 * st_n) // 4, st_n // 2

        def qk_units(mt):
            # K chunks first (pair mt's kt loop needs ALL of K), then Q chunks
            us = [(lambda cc=c: proj_qk_chunk(wk_sb, bk_sb, KT_sb, mt, cc, "k"))
                  for c in range(qc_n)]
            us += [(lambda cc=c: proj_qk_chunk(wq_sb, bq_sb, QT_sb, mt, cc, "q"))
                   for c in range(qc_n)]
            return us

        def make_fill(pr, units):
            # pop 2 worklist units per attention chunk (at kt==f1/f2); pair 0
            # additionally streams V during chunk 0 and Q0's later chunks;
            # pair 3 interleaves the out-projection of the previous chunk.
            def fill(c, kt):
                if pr == 0:
                    if c == 0:
                        proj_v(kt)
                    if kt == fmid and c + 1 < qc_n:
                        proj_qk_chunk(wq_sb, bq_sb, QT_sb, 0, c + 1, "q")
                if pr == MT - 1:
                    if kt == fmid and c >= 1:
                        outproj_chunk(c - 1)
                if kt in (f1, f2) and units:
                    units.pop(0)()
            return fill

        for pr in range(MT):
            units = qk_units(pr + 1) if pr + 1 < MT else []
            attention_pair(pr, make_fill(pr, units))
            assert not units, f"unemitted proj units after pair {pr}"
        outproj_chunk(qc_n - 1)

    nc.compile()
    return nc


def _part_major(a, inner):
    """[K*128, F] -> [128, K, F] with part[p, k, f] = a[k*128+p, f]."""
    k = a.shape[0] // P
    return np.ascontiguousarray(a.reshape(k, P, inner).transpose(1, 0, 2))


def make_in_maps(x, W_qkv, b_qkv, W_out):
    """Host-side sharding/permutation: per-core input dicts."""
    x = np.asarray(x, dtype=np.float32)
    W_qkv = np.asarray(W_qkv, dtype=np.float32)
    b_qkv = np.asarray(b_qkv, dtype=np.float32)
    W_out = np.asarray(W_out, dtype=np.float32)

    # per-head q/k/v rows of the fused projection
    Wh = W_qkv.reshape(H, 3 * HD, D)       # [16, 192, 1024]
    bh = b_qkv.reshape(H, 3 * HD)          # [16, 192]
    Wq_h, Wk_h, Wv_h = Wh[:, 0:HD], Wh[:, HD:2 * HD], Wh[:, 2 * HD:3 * HD]
    bq_h, bk_h, bv_h = bh[:, 0:HD], bh[:, HD:2 * HD], bh[:, 2 * HD:3 * HD]

    in_maps = []
    for core in range(8):
        b = core // 2
        g = core % 2
        hs = slice(8 * g, 8 * g + 8)

        xT = np.ascontiguousarray(x[b].T)                    # [1024, 2048]
        xT_dev = _part_major(xT, S).astype(BF16)             # [128, 8, 2048]

        Wq_core = Wq_h[hs].reshape(DC, D)                    # [512, 1024]
        Wk_core = Wk_h[hs].reshape(DC, D)
        Wv_core = Wv_h[hs].reshape(DC, D)
        wq_dev = _part_major(np.ascontiguousarray(Wq_core.T), DC).astype(BF16)
        wk_dev = _part_major(np.ascontiguousarray(Wk_core.T), DC).astype(BF16)
        wv_dev = _part_major(np.ascontiguousarray(Wv_core.T), DC).astype(BF16)

        Wo_core = W_out[:, DC * g:DC * (g + 1)]              # [1024, 512]
        wo_dev = _part_major(np.ascontiguousarray(Wo_core.T), D).astype(BF16)

        bq_dev = np.ascontiguousarray(
            bq_h[hs].reshape(DC).reshape(MT, P).T).astype(np.float32)
        bk_dev = np.ascontiguousarray(
            bk_h[hs].reshape(DC).reshape(MT, P).T).astype(np.float32)
        bv_dev = np.ascontiguousarray(
            bv_h[hs].reshape(DC).reshape(MT, P).T).astype(np.float32)

        in_maps.append({
            "xT": xT_dev, "wq": wq_dev, "wk": wk_dev, "wv": wv_dev,
            "wo": wo_dev, "bq": bq_dev, "bk": bk_dev, "bv": bv_dev,
        })
    return in_maps


def gather_out(results, b_out):
    """Sum the two per-batch partials, add bias, return [B, S, D] fp32."""
    b_out = np.asarray(b_out, dtype=np.float32)
    out = np.empty((B, S, D), np.float32)
    for b in range(B):
        part = results[2 * b]["out"] + results[2 * b + 1]["out"]   # [128, 8, 2048]
        outT = part.transpose(1, 0, 2).reshape(D, S)               # [1024, 2048]
        out[b] = outT.T + b_out
    return out


_NC_CACHE = {}


def run(x, W_qkv, b_qkv, W_out, b_out, trace=False):
    from concourse import bass_utils

    if "nc" not in _NC_CACHE:
        _NC_CACHE["nc"] = build_nc()
    nc = _NC_CACHE["nc"]

    in_maps = make_in_maps(x, W_qkv, b_qkv, W_out)
    res = bass_utils.run_bass_kernel_spmd(
        nc, in_maps, core_ids=list(range(8)), trace=trace
    )
    out = gather_out(res.results, b_out)
    return out, res


def kernel(x, W_qkv, b_qkv, W_out, b_out):
    out, _ = run(x, W_qkv, b_qkv, W_out, b_out, trace=False)
    return out


# revision 29
# speedup vs baseline: 1.1789x; 1.1789x over previous
# Multi-head attention (B=4, S=2048, D=1024, H=16) on 8 TRN2 NeuronCores.
#
# Sharding: core c handles batch b = c//2 and head-group g = c%2 (8 heads).
# Each core computes Q/K/V projections for its 8 heads, full attention over
# S=2048, and a partial output projection over its 512 value-features.
# Host sums the two partial outputs per batch and adds b_out.
#
# Device math (per core, all matmuls bf16 with fp32 PSUM accumulation):
#   Q^T, K^T  [512, S]   = W_slice @ x^T (+ per-partition bias)
#   V         [S, 512]   = x @ Wv^T  (bias folded into values, see below)
#   S^T tiles [128k, q]  = K_h Q_h^T, two heads row-tiled on the PE array
#   P^T       = exp(S^T * 0.125)                 (ACT engine, bf16 out)
#   [PV^T; l] = [V | 1]^T @ P^T                  (l = softmax denominator)
#   values^T  = PV^T * (1/l) + b_v               (exact: (P(V+b))/l = PV/l + b)
#   out^T     = W_out_slice^T-contraction over 512 features (partial)
import sys

for _p in ("/opt/trn_rl_repo",):
    if _p not in sys.path:
        sys.path.insert(0, _p)

import numpy as np
import ml_dtypes

BF16 = ml_dtypes.bfloat16

B, S, D = 4, 2048, 1024
H, HD = 16, 64
P = 128
HC = 8            # heads per core
DC = HC * HD      # 512 value-features per core
KS = D // P       # 8 contraction subtiles for projections
MT = DC // P      # 4 M-tiles == head pairs
ST = S // P       # 16 seq tiles
NQ = 512          # q-chunk (matmul free dim)
QC = S // NQ      # 4
OT = D // P       # 8 output e-tiles
OKS = DC // P     # 4 contraction subtiles for out-proj


def build_nc(s=S):
    """Build the single-core Bass/Tile program (SPMD across 8 cores)."""
    import concourse.tile as tile
    import concourse.mybir as mybir
    from concourse import bacc
    from contextlib import ExitStack

    dt = mybir.dt
    st_n = s // P
    qc_n = s // NQ

    nc = bacc.Bacc("TRN2", target_bir_lowering=False, debug=False)

    xT = nc.dram_tensor("xT", [P, KS, s], dt.bfloat16, kind="ExternalInput").ap()
    wq = nc.dram_tensor("wq", [P, KS, DC], dt.bfloat16, kind="ExternalInput").ap()
    wk = nc.dram_tensor("wk", [P, KS, DC], dt.bfloat16, kind="ExternalInput").ap()
    wv = nc.dram_tensor("wv", [P, KS, DC], dt.bfloat16, kind="ExternalInput").ap()
    wo = nc.dram_tensor("wo", [P, OKS, D], dt.bfloat16, kind="ExternalInput").ap()
    bq = nc.dram_tensor("bq", [P, MT], dt.float32, kind="ExternalInput").ap()
    bk = nc.dram_tensor("bk", [P, MT], dt.float32, kind="ExternalInput").ap()
    bv = nc.dram_tensor("bv", [P, MT], dt.float32, kind="ExternalInput").ap()
    out = nc.dram_tensor("out", [P, OT, s], dt.float32, kind="ExternalOutput").ap()

    with tile.TileContext(nc) as tc, ExitStack() as ctx:
        persist = ctx.enter_context(tc.tile_pool(name="persist", bufs=1))
        # PSUM budget is 8 banks: scores 2x[128,1024] (4) + one shared 4-slot
        # pool of 1-bank tiles for proj/pv/outproj accumulators (4).
        ps_s = ctx.enter_context(tc.tile_pool(name="ps_s", bufs=2, space="PSUM"))
        ps_pv = ctx.enter_context(tc.tile_pool(name="ps_pv", bufs=4, space="PSUM"))
        ps_io = ps_pv
        ptp = ctx.enter_context(tc.tile_pool(name="ptp", bufs=4))
        small = ctx.enter_context(tc.tile_pool(name="small", bufs=4))
        dramp = ctx.enter_context(tc.tile_pool(name="dramp", bufs=4, space="DRAM"))

        xT_sb = persist.tile([P, KS, s], dt.bfloat16, name="xT_sb")
        wq_sb = persist.tile([P, KS, DC], dt.bfloat16, name="wq_sb")
        wk_sb = persist.tile([P, KS, DC], dt.bfloat16, name="wk_sb")
        wv_sb = persist.tile([P, KS, DC], dt.bfloat16, name="wv_sb")
        wo_sb = persist.tile([P, OKS, D], dt.bfloat16, name="wo_sb")
        bq_sb = persist.tile([P, MT], dt.float32, name="bq_sb")
        bk_sb = persist.tile([P, MT], dt.float32, name="bk_sb")
        bv_sb = persist.tile([P, MT], dt.float32, name="bv_sb")
        QT_sb = persist.tile([P, MT, s], dt.bfloat16, name="QT_sb")
        KT_sb = persist.tile([P, MT, s], dt.bfloat16, name="KT_sb")
        V_sb = persist.tile([P, st_n, HC, HD + 1], dt.bfloat16, name="V_sb")
        VL_sb = persist.tile([P, OKS, s], dt.bfloat16, name="VL_sb")

        nc.sync.dma_start(out=wk_sb, in_=wk)
        for ks in range(KS):
            nc.sync.dma_start(out=xT_sb[:, ks], in_=xT[:, ks])
        nc.sync.dma_start(out=wq_sb, in_=wq)
        nc.sync.dma_start(out=bq_sb, in_=bq)
        nc.sync.dma_start(out=bk_sb, in_=bk)
        nc.sync.dma_start(out=wv_sb, in_=wv)
        nc.sync.dma_start(out=bv_sb, in_=bv)
        nc.sync.dma_start(out=wo_sb, in_=wo)
        # ones column for the fused softmax-denominator trick
        nc.vector.memset(V_sb[:, :, :, HD:HD + 1], 1.0)

        def proj_qk_chunk(w_sb, b_sb, dst, mt, c, wch):
            # dst[:, mt, c-chunk] = W_slice @ x^T + bias (128 rows = 2 heads)
            ps = ps_io.tile([P, NQ], dt.float32, name=f"ps_{wch}_{mt}_{c}", tag="pv")
            for ks in range(KS):
                nc.tensor.matmul(
                    ps,
                    lhsT=w_sb[:, ks, mt * P:(mt + 1) * P],
                    rhs=xT_sb[:, ks, c * NQ:(c + 1) * NQ],
                    start=(ks == 0),
                    stop=(ks == KS - 1),
                )
            nc.vector.tensor_add(
                dst[:, mt, c * NQ:(c + 1) * NQ],
                ps,
                b_sb[:, mt:mt + 1].to_broadcast((P, NQ)),
            )

        def proj_qk(w_sb, b_sb, dst, mt, wch):
            for c in range(qc_n):
                proj_qk_chunk(w_sb, b_sb, dst, mt, c, wch)

        def proj_v(st):
            # V[s-tile, :] = x @ Wv^T (no bias here; folded into values)
            ps = ps_io.tile([P, DC], dt.float32, name=f"ps_v_{st}", tag="pv")
            for ks in range(KS):
                nc.tensor.matmul(
                    ps,
                    lhsT=xT_sb[:, ks, st * P:(st + 1) * P],
                    rhs=wv_sb[:, ks, :],
                    start=(ks == 0),
                    stop=(ks == KS - 1),
                )
            nc.vector.tensor_copy(
                out=V_sb[:, st, :, 0:HD],
                in_=ps.rearrange("p (h d) -> p h d", h=HC),
            )

        def attention_pair(pr, filler=None):
            # filler(c, kt) emits interleaved PE work (projections for the
            # next pair, V tiles, out-proj chunks) so the ACT-bound kt loop
            # keeps the PE's slack cycles productive.
            for c in range(qc_n):
                cs = slice(c * NQ, (c + 1) * NQ)
                pvA = ps_pv.tile([HD + 1, NQ], dt.float32, name=f"pvA_{pr}_{c}", tag="pv")
                pvB = ps_pv.tile([HD + 1, NQ], dt.float32, name=f"pvB_{pr}_{c}", tag="pv")
                for kt in range(st_n):
                    if filler is not None:
                        filler(c, kt)
                    ks_sl = slice(kt * P, (kt + 1) * P)
                    ss = ps_s.tile([P, 2 * NQ], dt.float32, name=f"ss_{pr}_{c}_{kt}", tag="ss")
                    # S^T = K_h Q_h^T for the two heads, row-tiled (K=64 each)
                    nc.tensor.matmul(
                        ss[:, 0:NQ],
                        lhsT=KT_sb[0:HD, pr, ks_sl],
                        rhs=QT_sb[0:HD, pr, cs],
                    )
                    nc.tensor.matmul(
                        ss[:, NQ:2 * NQ],
                        lhsT=KT_sb[HD:P, pr, ks_sl],
                        rhs=QT_sb[HD:P, pr, cs],
                    )
                    pt = ptp.tile([P, 2 * NQ], dt.bfloat16, name=f"pt_{pr}_{c}_{kt}", tag="pt")
                    nc.scalar.activation(
                        pt, ss, mybir.ActivationFunctionType.Exp, scale=0.125
                    )
                    nc.tensor.matmul(
                        pvA,
                        lhsT=V_sb[:, kt, 2 * pr, :],
                        rhs=pt[:, 0:NQ],
                        start=(kt == 0),
                        stop=(kt == st_n - 1),
                    )
                    nc.tensor.matmul(
                        pvB,
                        lhsT=V_sb[:, kt, 2 * pr + 1, :],
                        rhs=pt[:, NQ:2 * NQ],
                        start=(kt == 0),
                        stop=(kt == st_n - 1),
                    )
                # Normalize: values^T = PV^T * (1/l) + b_v; head B is DMA-shifted
                # to partitions 64..127 so out-proj sees [128, s] rhs tiles.
                # First evacuate the PV psum accumulators to SBUF so the psum
                # banks free up in ~1us — the (slow) normalization chain then
                # runs off the critical path without stalling the next chunk.
                for half, pv in ((0, pvA), (1, pvB)):
                    pvs = small.tile([HD + 1, NQ], dt.float32, name=f"pvs_{pr}_{c}_{half}", tag="pvs")
                    nc.vector.tensor_copy(out=pvs, in_=pv)
                    # reciprocal of the l row, then DMA-broadcast it down to
                    # partitions 0..63 (SBUF APs can't have stride-0 partition
                    # dims, DRAM APs can: bounce through DRAM).
                    rec = small.tile([HD + 1, NQ], dt.float32, name=f"r_{pr}_{c}_{half}", tag="rec")
                    nc.vector.reciprocal(rec[HD:HD + 1, :], pvs[HD:HD + 1, :])
                    ld = dramp.tile([1, NQ], dt.float32, name=f"ld_{pr}_{c}_{half}", tag="ld")
                    nc.sync.dma_start(out=ld, in_=rec[HD:HD + 1, :])
                    bc = small.tile([HD, NQ], dt.float32, name=f"bc_{pr}_{c}_{half}", tag="bc")
                    nc.sync.dma_start(out=bc, in_=ld.to_broadcast((HD, NQ)))
                    if half == 0:
                        nc.vector.tensor_mul(VL_sb[0:HD, pr, cs], pvs[0:HD, :], bc)
                    else:
                        stg = small.tile([HD, NQ], dt.bfloat16, name=f"st_{pr}_{c}", tag="stg")
                        nc.vector.tensor_mul(stg, pvs[0:HD, :], bc)
                        nc.sync.dma_start(out=VL_sb[HD:P, pr, cs], in_=stg)
                nc.vector.tensor_add(
                    VL_sb[:, pr, cs],
                    VL_sb[:, pr, cs],
                    bv_sb[:, pr:pr + 1].to_broadcast((P, NQ)),
                )

        def outproj_chunk(c):
            # partial out-projection over this core's 512 value-features
            for et in range(OT):
                po = ps_io.tile([P, NQ], dt.float32, name=f"po_{et}_{c}", tag="pv")
                for ks in range(OKS):
                    nc.tensor.matmul(
                        po,
                        lhsT=wo_sb[:, ks, et * P:(et + 1) * P],
                        rhs=VL_sb[:, ks, c * NQ:(c + 1) * NQ],
                        start=(ks == 0),
                        stop=(ks == OKS - 1),
                    )
                ot_sb = small.tile([P, NQ], dt.float32, name=f"ot_{et}_{c}", tag="ot")
                nc.vector.tensor_copy(out=ot_sb, in_=po)
                nc.sync.dma_start(out=out[:, et, c * NQ:(c + 1) * NQ], in_=ot_sb)

        # Interleaved emission. Pair 0: K0 fully + Q0 chunk 0 up front; V
        # tiles stream inside chunk 0's kt loop; later Q0 chunks mid-chunk.
        proj_qk(wk_sb, bk_sb, KT_sb, 0, "k")
        proj_qk_chunk(wq_sb, bq_sb, QT_sb, 0, 0, "q")

        f1, f2, fmid = st_n // 4, (3 * st_n) // 4, st_n // 2

        def qk_units(mt):
            # K chunks first (pair mt's kt loop needs ALL of K), then Q chunks
            us = [(lambda cc=c: proj_qk_chunk(wk_sb, bk_sb, KT_sb, mt, cc, "k"))
                  for c in range(qc_n)]
            us += [(lambda cc=c: proj_qk_chunk(wq_sb, bq_sb, QT_sb, mt, cc, "q"))
                   for c in range(qc_n)]
            return us

        def make_fill(pr, units):
            # pop 2 worklist units per attention chunk (at kt==f1/f2); pair 0
            # additionally streams V during chunk 0 and Q0's later chunks;
            # pair 3 interleaves the out-projection of the previous chunk.
            def fill(c, kt):
                if pr == 0:
                    if c == 0:
                        proj_v(kt)
                    if kt == fmid and c + 1 < qc_n:
                        proj_qk_chunk(wq_sb, bq_sb, QT_sb, 0, c + 1, "q")
                if pr == MT - 1:
                    if kt == fmid and c >= 1:
                        outproj_chunk(c - 1)
                if kt in (f1, f2) and units:
                    units.pop(0)()
            return fill

        for pr in range(MT):
            units = qk_units(pr + 1) if pr + 1 < MT else []
            attention_pair(pr, make_fill(pr, units))
            assert not units, f"unemitted proj units after pair {pr}"
        outproj_chunk(qc_n - 1)

    nc.compile()
    return nc


def _part_major(a, inner):
    """[K*128, F] -> [128, K, F] with part[p, k, f] = a[k*128+p, f]."""
    k = a.shape[0] // P
    return np.ascontiguousarray(a.reshape(k, P, inner).transpose(1, 0, 2))


def make_in_maps(x, W_qkv, b_qkv, W_out):
    """Host-side sharding/permutation: per-core input dicts."""
    x = np.asarray(x, dtype=np.float32)
    W_qkv = np.asarray(W_qkv, dtype=np.float32)
    b_qkv = np.asarray(b_qkv, dtype=np.float32)
    W_out = np.asarray(W_out, dtype=np.float32)

    # per-head q/k/v rows of the fused projection
    Wh = W_qkv.reshape(H, 3 * HD, D)       # [16, 192, 1024]
    bh = b_qkv.reshape(H, 3 * HD)          # [16, 192]
    Wq_h, Wk_h, Wv_h = Wh[:, 0:HD], Wh[:, HD:2 * HD], Wh[:, 2 * HD:3 * HD]
    bq_h, bk_h, bv_h = bh[:, 0:HD], bh[:, HD:2 * HD], bh[:, 2 * HD:3 * HD]

    in_maps = []
    for core in range(8):
        b = core // 2
        g = core % 2
        hs = slice(8 * g, 8 * g + 8)

        xT = np.ascontiguousarray(x[b].T)                    # [1024, 2048]
        xT_dev = _part_major(xT, S).astype(BF16)             # [128, 8, 2048]

        Wq_core = Wq_h[hs].reshape(DC, D)                    # [512, 1024]
        Wk_core = Wk_h[hs].reshape(DC, D)
        Wv_core = Wv_h[hs].reshape(DC, D)
        wq_dev = _part_major(np.ascontiguousarray(Wq_core.T), DC).astype(BF16)
        wk_dev = _part_major(np.ascontiguousarray(Wk_core.T), DC).astype(BF16)
        wv_dev = _part_major(np.ascontiguousarray(Wv_core.T), DC).astype(BF16)

        Wo_core = W_out[:, DC * g:DC * (g + 1)]              # [1024, 512]
        wo_dev = _part_major(np.ascontiguousarray(Wo_core.T), D).astype(BF16)

        bq_dev = np.ascontiguousarray(
            bq_h[hs].reshape(DC).reshape(MT, P).T).astype(np.float32)
        bk_dev = np.ascontiguousarray(
            bk_h[hs].reshape(DC).reshape(MT, P).T).astype(np.float32)
        bv_dev = np.ascontiguousarray(
            bv_h[hs].reshape(DC).reshape(MT, P).T).astype(np.float32)

        in_maps.append({
            "xT": xT_dev, "wq": wq_dev, "wk": wk_dev, "wv": wv_dev,
            "wo": wo_dev, "bq": bq_dev, "bk": bk_dev, "bv": bv_dev,
        })
    return in_maps


def gather_out(results, b_out):
    """Sum the two per-batch partials, add bias, return [B, S, D] fp32."""
    b_out = np.asarray(b_out, dtype=np.float32)
    out = np.empty((B, S, D), np.float32)
    for b in range(B):
        part = results[2 * b]["out"] + results[2 * b + 1]["out"]   # [128, 8, 2048]
        outT = part.transpose(1, 0, 2).reshape(D, S)               # [1024, 2048]
        out[b] = outT.T + b_out
    return out


_NC_CACHE = {}


def run(x, W_qkv, b_qkv, W_out, b_out, trace=False):
    from concourse import bass_utils

    if "nc" not in _NC_CACHE:
        _NC_CACHE["nc"] = build_nc()
    nc = _NC_CACHE["nc"]

    in_maps = make_in_maps(x, W_qkv, b_qkv, W_out)
    res = bass_utils.run_bass_kernel_spmd(
        nc, in_maps, core_ids=list(range(8)), trace=trace
    )
    out = gather_out(res.results, b_out)
    return out, res


def kernel(x, W_qkv, b_qkv, W_out, b_out):
    out, _ = run(x, W_qkv, b_qkv, W_out, b_out, trace=False)
    return out


# revision 33
# speedup vs baseline: 1.1883x; 1.0080x over previous
# Multi-head attention (B=4, S=2048, D=1024, H=16) on 8 TRN2 NeuronCores.
#
# Sharding: core c handles batch b = c//2 and head-group g = c%2 (8 heads).
# Each core computes Q/K/V projections for its 8 heads, full attention over
# S=2048, and a partial output projection over its 512 value-features.
# Host sums the two partial outputs per batch and adds b_out.
#
# Device math (per core, all matmuls bf16 with fp32 PSUM accumulation):
#   Q^T, K^T  [512, S]   = W_slice @ x^T (+ per-partition bias)
#   V         [S, 512]   = x @ Wv^T  (bias folded into values, see below)
#   S^T tiles [128k, q]  = K_h Q_h^T, two heads row-tiled on the PE array
#   P^T       = exp(S^T * 0.125)                 (ACT engine, bf16 out)
#   [PV^T; l] = [V | 1]^T @ P^T                  (l = softmax denominator)
#   values^T  = PV^T * (1/l) + b_v               (exact: (P(V+b))/l = PV/l + b)
#   out^T     = W_out_slice^T-contraction over 512 features (partial)
import sys

for _p in ("/opt/trn_rl_repo",):
    if _p not in sys.path:
        sys.path.insert(0, _p)

import numpy as np
import ml_dtypes

BF16 = ml_dtypes.bfloat16

B, S, D = 4, 2048, 1024
H, HD = 16, 64
P = 128
HC = 8            # heads per core
DC = HC * HD      # 512 value-features per core
KS = D // P       # 8 contraction subtiles for projections
MT = DC // P      # 4 M-tiles == head pairs
ST = S // P       # 16 seq tiles
NQ = 512          # q-chunk (matmul free dim)
QC = S // NQ      # 4
OT = D // P       # 8 output e-tiles
OKS = DC // P     # 4 contraction subtiles for out-proj


def build_nc(s=S):
    """Build the single-core Bass/Tile program (SPMD across 8 cores)."""
    import concourse.tile as tile
    import concourse.mybir as mybir
    from concourse import bacc
    from contextlib import ExitStack

    dt = mybir.dt
    st_n = s // P
    qc_n = s // NQ

    nc = bacc.Bacc("TRN2", target_bir_lowering=False, debug=False)

    xT = nc.dram_tensor("xT", [P, KS, s], dt.bfloat16, kind="ExternalInput").ap()
    wq = nc.dram_tensor("wq", [P, KS, DC], dt.bfloat16, kind="ExternalInput").ap()
    wk = nc.dram_tensor("wk", [P, KS, DC], dt.bfloat16, kind="ExternalInput").ap()
    wv = nc.dram_tensor("wv", [P, KS, DC], dt.bfloat16, kind="ExternalInput").ap()
    wo = nc.dram_tensor("wo", [P, OKS, D], dt.bfloat16, kind="ExternalInput").ap()
    bq = nc.dram_tensor("bq", [P, MT], dt.float32, kind="ExternalInput").ap()
    bk = nc.dram_tensor("bk", [P, MT], dt.float32, kind="ExternalInput").ap()
    bv = nc.dram_tensor("bv", [P, MT], dt.float32, kind="ExternalInput").ap()
    out = nc.dram_tensor("out", [P, OT, s], dt.float32, kind="ExternalOutput").ap()

    with tile.TileContext(nc) as tc, ExitStack() as ctx:
        persist = ctx.enter_context(tc.tile_pool(name="persist", bufs=1))
        # PSUM budget is 8 banks: scores 2x[128,1024] (4) + one shared 4-slot
        # pool of 1-bank tiles for proj/pv/outproj accumulators (4).
        ps_s = ctx.enter_context(tc.tile_pool(name="ps_s", bufs=2, space="PSUM"))
        ps_pv = ctx.enter_context(tc.tile_pool(name="ps_pv", bufs=4, space="PSUM"))
        ps_io = ps_pv
        ptp = ctx.enter_context(tc.tile_pool(name="ptp", bufs=4))
        small = ctx.enter_context(tc.tile_pool(name="small", bufs=4))
        dramp = ctx.enter_context(tc.tile_pool(name="dramp", bufs=4, space="DRAM"))

        xT_sb = persist.tile([P, KS, s], dt.bfloat16, name="xT_sb")
        wq_sb = persist.tile([P, KS, DC], dt.bfloat16, name="wq_sb")
        wk_sb = persist.tile([P, KS, DC], dt.bfloat16, name="wk_sb")
        wv_sb = persist.tile([P, KS, DC], dt.bfloat16, name="wv_sb")
        wo_sb = persist.tile([P, OKS, D], dt.bfloat16, name="wo_sb")
        bq_sb = persist.tile([P, MT], dt.float32, name="bq_sb")
        bk_sb = persist.tile([P, MT], dt.float32, name="bk_sb")
        bv_sb = persist.tile([P, MT], dt.float32, name="bv_sb")
        QT_sb = persist.tile([P, MT, s], dt.bfloat16, name="QT_sb")
        KT_sb = persist.tile([P, MT, s], dt.bfloat16, name="KT_sb")
        V_sb = persist.tile([P, st_n, HC, HD + 1], dt.bfloat16, name="V_sb")
        VL_sb = persist.tile([P, OKS, s], dt.bfloat16, name="VL_sb")

        for ks in range(KS):
            nc.sync.dma_start(out=wk_sb[:, ks], in_=wk[:, ks])
        for ks in range(KS):
            nc.sync.dma_start(out=xT_sb[:, ks], in_=xT[:, ks])
        nc.sync.dma_start(out=wq_sb, in_=wq)
        nc.sync.dma_start(out=bq_sb, in_=bq)
        nc.sync.dma_start(out=bk_sb, in_=bk)
        nc.sync.dma_start(out=wv_sb, in_=wv)
        nc.sync.dma_start(out=bv_sb, in_=bv)
        nc.sync.dma_start(out=wo_sb, in_=wo)
        # ones column for the fused softmax-denominator trick
        nc.vector.memset(V_sb[:, :, :, HD:HD + 1], 1.0)

        def proj_qk_chunk(w_sb, b_sb, dst, mt, c, wch):
            # dst[:, mt, c-chunk] = W_slice @ x^T + bias (128 rows = 2 heads)
            ps = ps_io.tile([P, NQ], dt.float32, name=f"ps_{wch}_{mt}_{c}", tag="pv")
            for ks in range(KS):
                nc.tensor.matmul(
                    ps,
                    lhsT=w_sb[:, ks, mt * P:(mt + 1) * P],
                    rhs=xT_sb[:, ks, c * NQ:(c + 1) * NQ],
                    start=(ks == 0),
                    stop=(ks == KS - 1),
                )
            nc.vector.tensor_add(
                dst[:, mt, c * NQ:(c + 1) * NQ],
                ps,
                b_sb[:, mt:mt + 1].to_broadcast((P, NQ)),
            )

        def proj_qk(w_sb, b_sb, dst, mt, wch):
            for c in range(qc_n):
                proj_qk_chunk(w_sb, b_sb, dst, mt, c, wch)

        def proj_v(st):
            # V[s-tile, :] = x @ Wv^T (no bias here; folded into values)
            ps = ps_io.tile([P, DC], dt.float32, name=f"ps_v_{st}", tag="pv")
            for ks in range(KS):
                nc.tensor.matmul(
                    ps,
                    lhsT=xT_sb[:, ks, st * P:(st + 1) * P],
                    rhs=wv_sb[:, ks, :],
                    start=(ks == 0),
                    stop=(ks == KS - 1),
                )
            nc.vector.tensor_copy(
                out=V_sb[:, st, :, 0:HD],
                in_=ps.rearrange("p (h d) -> p h d", h=HC),
            )

        def attention_pair(pr, filler=None):
            # filler(c, kt) emits interleaved PE work (projections for the
            # next pair, V tiles, out-proj chunks) so the ACT-bound kt loop
            # keeps the PE's slack cycles productive.
            for c in range(qc_n):
                cs = slice(c * NQ, (c + 1) * NQ)
                pvA = ps_pv.tile([HD + 1, NQ], dt.float32, name=f"pvA_{pr}_{c}", tag="pv")
                pvB = ps_pv.tile([HD + 1, NQ], dt.float32, name=f"pvB_{pr}_{c}", tag="pv")
                for kt in range(st_n):
                    if filler is not None:
                        filler(c, kt)
                    ks_sl = slice(kt * P, (kt + 1) * P)
                    ss = ps_s.tile([P, 2 * NQ], dt.float32, name=f"ss_{pr}_{c}_{kt}", tag="ss")
                    # S^T = K_h Q_h^T for the two heads, row-tiled (K=64 each)
                    nc.tensor.matmul(
                        ss[:, 0:NQ],
                        lhsT=KT_sb[0:HD, pr, ks_sl],
                        rhs=QT_sb[0:HD, pr, cs],
                    )
                    nc.tensor.matmul(
                        ss[:, NQ:2 * NQ],
                        lhsT=KT_sb[HD:P, pr, ks_sl],
                        rhs=QT_sb[HD:P, pr, cs],
                    )
                    pt = ptp.tile([P, 2 * NQ], dt.bfloat16, name=f"pt_{pr}_{c}_{kt}", tag="pt")
                    nc.scalar.activation(
                        pt, ss, mybir.ActivationFunctionType.Exp, scale=0.125
                    )
                    nc.tensor.matmul(
                        pvA,
                        lhsT=V_sb[:, kt, 2 * pr, :],
                        rhs=pt[:, 0:NQ],
                        start=(kt == 0),
                        stop=(kt == st_n - 1),
                    )
                    nc.tensor.matmul(
                        pvB,
                        lhsT=V_sb[:, kt, 2 * pr + 1, :],
                        rhs=pt[:, NQ:2 * NQ],
                        start=(kt == 0),
                        stop=(kt == st_n - 1),
                    )
                # Normalize: values^T = PV^T * (1/l) + b_v; head B is DMA-shifted
                # to partitions 64..127 so out-proj sees [128, s] rhs tiles.
                # First evacuate the PV psum accumulators to SBUF so the psum
                # banks free up in ~1us — the (slow) normalization chain then
                # runs off the critical path without stalling the next chunk.
                for half, pv in ((0, pvA), (1, pvB)):
                    pvs = small.tile([HD + 1, NQ], dt.float32, name=f"pvs_{pr}_{c}_{half}", tag="pvs")
                    nc.vector.tensor_copy(out=pvs, in_=pv)
                    # reciprocal of the l row, then DMA-broadcast it down to
                    # partitions 0..63 (SBUF APs can't have stride-0 partition
                    # dims, DRAM APs can: bounce through DRAM).
                    rec = small.tile([HD + 1, NQ], dt.float32, name=f"r_{pr}_{c}_{half}", tag="rec")
                    nc.vector.reciprocal(rec[HD:HD + 1, :], pvs[HD:HD + 1, :])
                    ld = dramp.tile([1, NQ], dt.float32, name=f"ld_{pr}_{c}_{half}", tag="ld")
                    nc.sync.dma_start(out=ld, in_=rec[HD:HD + 1, :])
                    bc = small.tile([HD, NQ], dt.float32, name=f"bc_{pr}_{c}_{half}", tag="bc")
                    nc.sync.dma_start(out=bc, in_=ld.to_broadcast((HD, NQ)))
                    if half == 0:
                        nc.vector.tensor_mul(VL_sb[0:HD, pr, cs], pvs[0:HD, :], bc)
                    else:
                        stg = small.tile([HD, NQ], dt.bfloat16, name=f"st_{pr}_{c}", tag="stg")
                        nc.vector.tensor_mul(stg, pvs[0:HD, :], bc)
                        nc.sync.dma_start(out=VL_sb[HD:P, pr, cs], in_=stg)
                nc.vector.tensor_add(
                    VL_sb[:, pr, cs],
                    VL_sb[:, pr, cs],
                    bv_sb[:, pr:pr + 1].to_broadcast((P, NQ)),
                )

        def outproj_et(c, et):
            # partial out-projection over this core's 512 value-features
            po = ps_io.tile([P, NQ], dt.float32, name=f"po_{et}_{c}", tag="pv")
            for ks in range(OKS):
                nc.tensor.matmul(
                    po,
                    lhsT=wo_sb[:, ks, et * P:(et + 1) * P],
                    rhs=VL_sb[:, ks, c * NQ:(c + 1) * NQ],
                    start=(ks == 0),
                    stop=(ks == OKS - 1),
                )
            ot_sb = small.tile([P, NQ], dt.float32, name=f"ot_{et}_{c}", tag="ot")
            nc.vector.tensor_copy(out=ot_sb, in_=po)
            nc.sync.dma_start(out=out[:, et, c * NQ:(c + 1) * NQ], in_=ot_sb)

        def outproj_chunk(c):
            for et in range(OT):
                outproj_et(c, et)

        # Interleaved emission. Pair 0: K0 fully + Q0 chunk 0 up front; V
        # tiles stream inside chunk 0's kt loop; later Q0 chunks mid-chunk.
        proj_qk(wk_sb, bk_sb, KT_sb, 0, "k")
        proj_qk_chunk(wq_sb, bq_sb, QT_sb, 0, 0, "q")

        f1, f2, fmid = st_n // 4, (3 * st_n) // 4, st_n // 2

        def qk_units(mt):
            # K chunks first (pair mt's kt loop needs ALL of K), then Q chunks
            us = [(lambda cc=c: proj_qk_chunk(wk_sb, bk_sb, KT_sb, mt, cc, "k"))
                  for c in range(qc_n)]
            us += [(lambda cc=c: proj_qk_chunk(wq_sb, bq_sb, QT_sb, mt, cc, "q"))
                   for c in range(qc_n)]
            return us

        def make_fill(pr, units):
            # pop 2 worklist units per attention chunk (at kt==f1/f2); pair 0
            # additionally streams V during chunk 0 and Q0's later chunks;
            # pair 3 interleaves the out-projection of the previous chunk.
            def fill(c, kt):
                if pr == 0:
                    if c == 0:
                        proj_v(kt)
                    if kt == fmid and c + 1 < qc_n:
                        proj_qk_chunk(wq_sb, bq_sb, QT_sb, 0, c + 1, "q")
                if pr == MT - 1 and c >= 1:
                    # previous chunk's out-projection, one e-tile at a time
                    if st_n >= 2 * OT:
                        if kt % 2 == 0:
                            outproj_et(c - 1, kt // 2)
                    elif kt == fmid:
                        outproj_chunk(c - 1)
                if kt in (f1, f2) and units:
                    units.pop(0)()
            return fill

        for pr in range(MT):
            units = qk_units(pr + 1) if pr + 1 < MT else []
            attention_pair(pr, make_fill(pr, units))
            assert not units, f"unemitted proj units after pair {pr}"
        outproj_chunk(qc_n - 1)

    nc.compile()
    return nc


def _part_major(a, inner):
    """[K*128, F] -> [128, K, F] with part[p, k, f] = a[k*128+p, f]."""
    k = a.shape[0] // P
    return np.ascontiguousarray(a.reshape(k, P, inner).transpose(1, 0, 2))


def make_in_maps(x, W_qkv, b_qkv, W_out):
    """Host-side sharding/permutation: per-core input dicts."""
    x = np.asarray(x, dtype=np.float32)
    W_qkv = np.asarray(W_qkv, dtype=np.float32)
    b_qkv = np.asarray(b_qkv, dtype=np.float32)
    W_out = np.asarray(W_out, dtype=np.float32)

    # per-head q/k/v rows of the fused projection
    Wh = W_qkv.reshape(H, 3 * HD, D)       # [16, 192, 1024]
    bh = b_qkv.reshape(H, 3 * HD)          # [16, 192]
    Wq_h, Wk_h, Wv_h = Wh[:, 0:HD], Wh[:, HD:2 * HD], Wh[:, 2 * HD:3 * HD]
    bq_h, bk_h, bv_h = bh[:, 0:HD], bh[:, HD:2 * HD], bh[:, 2 * HD:3 * HD]

    in_maps = []
    for core in range(8):
        b = core // 2
        g = core % 2
        hs = slice(8 * g, 8 * g + 8)

        xT = np.ascontiguousarray(x[b].T)                    # [1024, 2048]
        xT_dev = _part_major(xT, S).astype(BF16)             # [128, 8, 2048]

        Wq_core = Wq_h[hs].reshape(DC, D)                    # [512, 1024]
        Wk_core = Wk_h[hs].reshape(DC, D)
        Wv_core = Wv_h[hs].reshape(DC, D)
        wq_dev = _part_major(np.ascontiguousarray(Wq_core.T), DC).astype(BF16)
        wk_dev = _part_major(np.ascontiguousarray(Wk_core.T), DC).astype(BF16)
        wv_dev = _part_major(np.ascontiguousarray(Wv_core.T), DC).astype(BF16)

        Wo_core = W_out[:, DC * g:DC * (g + 1)]              # [1024, 512]
        wo_dev = _part_major(np.ascontiguousarray(Wo_core.T), D).astype(BF16)

        bq_dev = np.ascontiguousarray(
            bq_h[hs].reshape(DC).reshape(MT, P).T).astype(np.float32)
        bk_dev = np.ascontiguousarray(
            bk_h[hs].reshape(DC).reshape(MT, P).T).astype(np.float32)
        bv_dev = np.ascontiguousarray(
            bv_h[hs].reshape(DC).reshape(MT, P).T).astype(np.float32)

        in_maps.append({
            "xT": xT_dev, "wq": wq_dev, "wk": wk_dev, "wv": wv_dev,
            "wo": wo_dev, "bq": bq_dev, "bk": bk_dev, "bv": bv_dev,
        })
    return in_maps


def gather_out(results, b_out):
    """Sum the two per-batch partials, add bias, return [B, S, D] fp32."""
    b_out = np.asarray(b_out, dtype=np.float32)
    out = np.empty((B, S, D), np.float32)
    for b in range(B):
        part = results[2 * b]["out"] + results[2 * b + 1]["out"]   # [128, 8, 2048]
        outT = part.transpose(1, 0, 2).reshape(D, S)               # [1024, 2048]
        out[b] = outT.T + b_out
    return out


_NC_CACHE = {}


def run(x, W_qkv, b_qkv, W_out, b_out, trace=False):
    from concourse import bass_utils

    if "nc" not in _NC_CACHE:
        _NC_CACHE["nc"] = build_nc()
    nc = _NC_CACHE["nc"]

    in_maps = make_in_maps(x, W_qkv, b_qkv, W_out)
    res = bass_utils.run_bass_kernel_spmd(
        nc, in_maps, core_ids=list(range(8)), trace=trace
    )
    out = gather_out(res.results, b_out)
    return out, res


def kernel(x, W_qkv, b_qkv, W_out, b_out):
    out, _ = run(x, W_qkv, b_qkv, W_out, b_out, trace=False)
    return out
